# revision 27
# baseline (speedup 1.0000x reference)
"""Trainium2 Bass kernel for nn_Dilation2D (101x101 grayscale dilation with a
parabolic structuring element).

Math: out[r, c] = max_{i,j} padded[i + c, j + r] + h[i, j] with
h[i, j] = -(z_i^2 + z_j^2) / (4 s) separable into f(i) + g(j), so the 2D
max-plus convolution factors into two 1D sliding passes:

  stage 1:  t[p, r] = max_j rowpad[p, j + r] + w[j]     (slide along columns)
  stage 2:  out[r, c] = max_i tpad[i + c, r] + w[i]     (slide along rows)

with w[k] = -(k - 50)^2 / (4 s) and sentinel (-1e30) padding instead of -inf.

Sharding: output rows are split across the 8 cores (13 rows each, 104 >= 101).
Each core runs both stages restricted to its 13 output rows -- no cross-core
communication. Stage 1 keeps input rows on partitions (101 used): one
broadcast-add (tensor_tensor over a [101, 13, 101] sliding-window AP) plus a
free-dim max-reduce. The [101, 13] result is transposed on the tensor engine,
sentinel-padded to [13, 224], and replicated into a [104, 128] layout
(partition P = cc*13 + r holds tpad[r, cc*13 : cc*13+128]) so stage 2 is
again one broadcast-add + free-dim max-reduce across 104 partitions.

Implementation is raw Bass (no Tile framework): manual semaphores avoid the
Tile entry/exit barrier overhead (~12 us on this toolchain), and all eight
replication gathers increment one shared semaphore so the single-sem-wait
ISA limit is satisfied with standalone wait instructions. The transpose
identity is built on-chip by gpsimd; w arrives pre-replicated from the host.
The replication gathers are spread over all three DMA issuers (SP HWDGE,
ACT HWDGE, and gpsimd SWDGE) so three descriptor generators run in parallel.
"""

import numpy as np

K = 101          # image/kernel size
PAD = 50
S = 13           # output rows per core
NCORES = 8
W = S + K - 1    # 113: window columns each core needs for compute
WT = 128         # transfer width: 512-byte rows
XCOLS = 224      # host-side padded row length (>= 7*13 + 128)
TCOLS = 224      # stage-2 padded t row length (>= 7*13 + 128)
SENT = np.float32(-1.0e30)

_CACHE = {}


def _build_nc():
    import concourse.bass as bass
    import concourse.mybir as mybir

    f32 = mybir.dt.float32
    add = mybir.AluOpType.add
    amax = mybir.AluOpType.max

    class _FastBass(bass.Bass):
        # Bass.__init__ ends with an all-engine barrier that separates the
        # const-tensor memsets from user code; this kernel uses none of the
        # const tensors and every cross-engine handoff is semaphore-guarded,
        # so the barrier only adds ~0.8 us of startup. Skip it during
        # construction only.
        def all_engine_barrier(self):
            if getattr(self, "_in_init", True):
                return None
            return super().all_engine_barrier()

    nc = _FastBass(target_bir_lowering=False, debug=False, enable_asserts=False)

    x_in = nc.dram_tensor("x", [K, WT], f32, kind="ExternalInput")
    w_in = nc.dram_tensor("w", [NCORES * S, K], f32, kind="ExternalInput")
    out = nc.dram_tensor("out", [NCORES * S, S], f32, kind="ExternalOutput")

    with (
        nc.sbuf_tensor("xs", [K, WT], f32) as xs,
        nc.sbuf_tensor("wsb", [NCORES * S, K], f32) as wsb,
        nc.sbuf_tensor("ones_k", [K, K], f32) as ones_k,
        nc.sbuf_tensor("idn", [K, K], f32) as idn,
        nc.sbuf_tensor("tmp1", [K, S * K], f32) as tmp1,
        nc.sbuf_tensor("t1", [K, S], f32) as t1,
        nc.sbuf_tensor("tpad", [S, TCOLS], f32) as tpad,
        nc.sbuf_tensor("X", [NCORES * S, WT], f32) as X,
        nc.sbuf_tensor("tmp2", [NCORES * S, S * K], f32) as tmp2,
        nc.sbuf_tensor("osb", [NCORES * S, S], f32) as osb,
        nc.psum_tensor("tp_ps", [S, K], f32) as tp_ps,
        nc.semaphore("s_dx") as s_dx,
        nc.semaphore("s_dx2") as s_dx2,
        nc.semaphore("s_dw") as s_dw,
        nc.semaphore("s_idn") as s_idn,
        nc.semaphore("s_pe") as s_pe,
        nc.semaphore("s_dve") as s_dve,
        nc.semaphore("s_g") as s_g,
nc.semaphore("s_g2") as s_g2,
        nc.semaphore("s_out") as s_out,
        nc.Block() as block,
    ):
        xs_win = bass.AP(xs, 0, [[WT, K], [1, S], [1, K]])
        ws_b1 = bass.AP(wsb, 0, [[K, K], [0, S], [1, K]])
        tmp1_w = bass.AP(tmp1, 0, [[S * K, K], [K, S], [1, K]])
        X_win = bass.AP(X, 0, [[WT, NCORES * S], [1, S], [1, K]])
        ws_b2 = bass.AP(wsb, 0, [[K, NCORES * S], [0, S], [1, K]])
        tmp2_w = bass.AP(tmp2, 0, [[S * K, NCORES * S], [K, S], [1, K]])

        def gather(eng, cc, sem):
            return eng.dma_start(
                X[cc * S : (cc + 1) * S, :],
                tpad[0:S, cc * S : cc * S + WT],
                single_packet=True,
            ).then_inc(sem, 16)

        @block.sync
        def _(sync):
            # x rows 45..73 then w rows 0..52
            sync.dma_start(
                bass.AP(xs, 45 * WT, [[WT, 28], [1, WT]]),
                bass.AP(x_in, 45 * WT, [[WT, 28], [1, WT]]),
            ).then_inc(s_dx, 16)
            sync.dma_start(
                bass.AP(wsb, 0, [[K, 52], [1, K]]),
                bass.AP(w_in, 0, [[K, 52], [1, K]]),
            ).then_inc(s_dw, 16)
            sync.wait_ge(s_dve, 2)
            for cc in range(3):
                gather(sync, cc, s_g)

        @block.scalar
        def _(scalar):
            # x rows 73..101 then w rows 52..104
            scalar.dma_start(
                bass.AP(xs, 73 * WT, [[WT, 28], [1, WT]]),
                bass.AP(x_in, 73 * WT, [[WT, 28], [1, WT]]),
            ).then_inc(s_dx, 16)
            scalar.dma_start(
                bass.AP(wsb, 52 * K, [[K, 52], [1, K]]),
                bass.AP(w_in, 52 * K, [[K, 52], [1, K]]),
            ).then_inc(s_dw, 16)
            scalar.wait_ge(s_dve, 2)
            for cc in range(3, 6):
                gather(scalar, cc, s_g)
            scalar.wait_ge(s_dve, 3)
            scalar.dma_start(out[:, :], osb[:, :]).then_inc(s_out, 16)

        @block.gpsimd
        def _(gpsimd):
            gpsimd.dma_start(
                bass.AP(xs, 0, [[WT, 45], [1, WT]]),
                bass.AP(x_in, 0, [[WT, 45], [1, WT]]),
            ).then_inc(s_dx2, 16)
            gpsimd.memset(ones_k[:, :], 1.0)
            gpsimd.drain()
            gpsimd.affine_select(
                idn[:, :],
                ones_k[:, :],
                [[1, K]],
                mybir.AluOpType.is_equal,
                0.0,
                base=0,
                channel_multiplier=-1,
            ).then_inc(s_idn, 1)
            gpsimd.wait_ge(s_dve, 2)
            for cc in range(6, NCORES):
                gather(gpsimd, cc, s_g2)

        @block.tensor
        def _(tensor):
            tensor.wait_ge(s_idn, 1)
            tensor.wait_ge(s_dve, 1)
            tensor.transpose(tp_ps[:, :], t1[:, :], idn[:, :]).then_inc(s_pe, 1)

        @block.vector
        def _(vector):
            vector.memset(tpad[:, :], float(SENT))
            vector.wait_ge(s_dw, 32)
            vector.wait_ge(s_dx, 32)
            vector.wait_ge(s_dx2, 16)
            # stage 1: tmp1[p, r, j] = xs[p, r + j] + w[j]
            vector.tensor_tensor(tmp1_w, xs_win, ws_b1, add)
            vector.drain()
            vector.tensor_reduce(
                t1[:, :], tmp1_w, axis=mybir.AxisListType.X, op=amax
            ).then_inc(s_dve, 1)
            vector.wait_ge(s_pe, 1)
            vector.drain()
            # tpad[r, 50 + p] = t1[p, r]
            vector.tensor_copy(tpad[0:S, PAD : PAD + K], tp_ps[:, :]).then_inc(
                s_dve, 1
            )
            vector.wait_ge(s_g, 96)
            vector.wait_ge(s_g2, 32)
            # stage 2: tmp2[P, c, i] = X[P, c + i] + w[i]
            vector.tensor_tensor(tmp2_w, X_win, ws_b2, add)
            vector.drain()
            vector.tensor_reduce(
                osb[:, :], tmp2_w, axis=mybir.AxisListType.X, op=amax
            ).then_inc(s_dve, 1)

    # restore normal barrier behavior for any framework-emitted code that
    # runs after the block (the skipped barriers are the init and block-exit
    # ones; the BSP postamble still drains all queues before NEFF end)
    nc._in_init = False
    return nc


def _prep_in_maps(input, scale):
    inp = np.asarray(input, dtype=np.float32)
    s = np.float32(np.asarray(scale).reshape(()))

    z = (np.arange(K, dtype=np.float32) - np.float32(PAD)).astype(np.float32)
    zsq = (z * z).astype(np.float32)
    wvec = (-zsq / (np.float32(4.0) * s)).astype(np.float32)
    w_rep = np.ascontiguousarray(np.tile(wvec[None, :], (NCORES * S, 1)))

    rowpad = np.full((K, XCOLS), SENT, dtype=np.float32)
    rowpad[:, PAD : PAD + K] = inp

    in_maps = []
    for k in range(NCORES):
        in_maps.append(
            {
                "x": np.ascontiguousarray(rowpad[:, S * k : S * k + WT]),
                "w": w_rep,
            }
        )
    return in_maps


def _unshard(results):
    out_full = np.empty((K, K), dtype=np.float32)
    for k, res in enumerate(results):
        o = np.asarray(res["out"]).reshape(NCORES, S, S)  # [cc, r_loc, c_in]
        block = o.transpose(1, 0, 2).reshape(S, NCORES * S)  # [r_loc, c]
        r0 = S * k
        nrows = min(S, K - r0)
        if nrows <= 0:
            continue
        out_full[r0 : r0 + nrows, :] = block[:nrows, :K]
    return out_full


def kernel(input, scale):
    from concourse.bass_utils import run_bass_kernel_spmd

    if "nc" not in _CACHE:
        _CACHE["nc"] = _build_nc()
    nc = _CACHE["nc"]

    in_maps = _prep_in_maps(input, scale)
    res = run_bass_kernel_spmd(nc, in_maps, core_ids=list(range(NCORES)))
    return _unshard(res.results)


# revision 28
# speedup vs baseline: 1.0468x; 1.0468x over previous
"""Trainium2 Bass kernel for nn_Dilation2D (101x101 grayscale dilation with a
parabolic structuring element).

Math: out[r, c] = max_{i,j} padded[i + c, j + r] + h[i, j] with
h[i, j] = -(z_i^2 + z_j^2) / (4 s) separable into f(i) + g(j), so the 2D
max-plus convolution factors into two 1D sliding passes:

  stage 1:  t[p, r] = max_j rowpad[p, j + r] + w[j]     (slide along columns)
  stage 2:  out[r, c] = max_i tpad[i + c, r] + w[i]     (slide along rows)

with w[k] = -(k - 50)^2 / (4 s) and sentinel (-1e30) padding instead of -inf.

Sharding: output rows are split across the 8 cores (13 rows each, 104 >= 101).
Each core runs both stages restricted to its 13 output rows -- no cross-core
communication. Stage 1 keeps input rows on partitions (101 used): one
broadcast-add (tensor_tensor over a [101, 13, 101] sliding-window AP) plus a
free-dim max-reduce. The [101, 13] result is transposed on the tensor engine,
sentinel-padded to [13, 224], and replicated into a [104, 128] layout
(partition P = cc*13 + r holds tpad[r, cc*13 : cc*13+128]) so stage 2 is
again one broadcast-add + free-dim max-reduce across 104 partitions.

Implementation is raw Bass (no Tile framework): manual semaphores avoid the
Tile entry/exit barrier overhead (~12 us on this toolchain), and all eight
replication gathers increment one shared semaphore so the single-sem-wait
ISA limit is satisfied with standalone wait instructions. The transpose
identity is built on-chip by gpsimd; w arrives pre-replicated from the host.
The replication gathers are spread over all three DMA issuers (SP HWDGE,
ACT HWDGE, and gpsimd SWDGE) so three descriptor generators run in parallel.
"""

import numpy as np

K = 101          # image/kernel size
PAD = 50
S = 13           # output rows per core
NCORES = 8
W = S + K - 1    # 113: window columns each core needs for compute
WT = 128         # transfer width: 512-byte rows
XCOLS = 224      # host-side padded row length (>= 7*13 + 128)
TCOLS = 224      # stage-2 padded t row length (>= 7*13 + 128)
SENT = np.float32(-1.0e30)

_CACHE = {}


def _build_nc():
    import concourse.bass as bass
    import concourse.mybir as mybir

    f32 = mybir.dt.float32
    add = mybir.AluOpType.add
    amax = mybir.AluOpType.max

    class _FastBass(bass.Bass):
        # Bass.__init__ ends with an all-engine barrier that separates the
        # const-tensor memsets from user code; this kernel uses none of the
        # const tensors and every cross-engine handoff is semaphore-guarded,
        # so the barrier only adds ~0.8 us of startup. Skip it during
        # construction only.
        def all_engine_barrier(self):
            if getattr(self, "_in_init", True):
                return None
            return super().all_engine_barrier()

    nc = _FastBass(target_bir_lowering=False, debug=False, enable_asserts=False)

    x_in = nc.dram_tensor("x", [K, WT], f32, kind="ExternalInput")
    w_in = nc.dram_tensor("w", [NCORES * S, K], f32, kind="ExternalInput")
    out = nc.dram_tensor("out", [NCORES * S, S], f32, kind="ExternalOutput")

    with (
        nc.sbuf_tensor("xs", [K, WT], f32) as xs,
        nc.sbuf_tensor("wsb", [NCORES * S, K], f32) as wsb,
        nc.sbuf_tensor("ones_k", [K, K], f32) as ones_k,
        nc.sbuf_tensor("idn", [K, K], f32) as idn,
        nc.sbuf_tensor("tmp1", [K, S * K], f32) as tmp1,
        nc.sbuf_tensor("t1", [K, S], f32) as t1,
        nc.sbuf_tensor("tpad", [S, TCOLS], f32) as tpad,
        nc.sbuf_tensor("X", [NCORES * S, WT], f32) as X,
        nc.sbuf_tensor("tmp2", [NCORES * S, S * K], f32) as tmp2,
        nc.sbuf_tensor("osb", [NCORES * S, S], f32) as osb,
        nc.psum_tensor("tp_ps", [S, K], f32) as tp_ps,
        nc.semaphore("s_dx") as s_dx,
        nc.semaphore("s_dw") as s_dw,
        nc.semaphore("s_idn") as s_idn,
        nc.semaphore("s_pe") as s_pe,
        nc.semaphore("s_dve") as s_dve,
        nc.semaphore("s_g") as s_g,
nc.semaphore("s_g2") as s_g2,
        nc.semaphore("s_out") as s_out,
        nc.Block() as block,
    ):
        xs_win = bass.AP(xs, 0, [[WT, K], [1, S], [1, K]])
        ws_b1 = bass.AP(wsb, 0, [[K, K], [0, S], [1, K]])
        tmp1_w = bass.AP(tmp1, 0, [[S * K, K], [K, S], [1, K]])
        X_win = bass.AP(X, 0, [[WT, NCORES * S], [1, S], [1, K]])
        ws_b2 = bass.AP(wsb, 0, [[K, NCORES * S], [0, S], [1, K]])
        tmp2_w = bass.AP(tmp2, 0, [[S * K, NCORES * S], [K, S], [1, K]])

        def gather(eng, cc, sem):
            return eng.dma_start(
                X[cc * S : (cc + 1) * S, :],
                tpad[0:S, cc * S : cc * S + WT],
                single_packet=True,
            ).then_inc(sem, 16)

        @block.sync
        def _(sync):
            sync.dma_start(
                bass.AP(xs, 0, [[WT, 51], [1, WT]]),
                bass.AP(x_in, 0, [[WT, 51], [1, WT]]),
            ).then_inc(s_dx, 16)
            sync.dma_start(
                bass.AP(xs, 51 * WT, [[WT, 50], [1, WT]]),
                bass.AP(x_in, 51 * WT, [[WT, 50], [1, WT]]),
            ).then_inc(s_dx, 16)
            sync.wait_ge(s_dve, 2)
            for cc in range(3):
                gather(sync, cc, s_g)

        @block.scalar
        def _(scalar):
            scalar.dma_start(wsb[:, :], w_in[:, :]).then_inc(s_dw, 16)
            scalar.wait_ge(s_dve, 2)
            for cc in range(3, 6):
                gather(scalar, cc, s_g)
            scalar.wait_ge(s_dve, 3)
            scalar.dma_start(out[:, :], osb[:, :]).then_inc(s_out, 16)

        @block.gpsimd
        def _(gpsimd):
            gpsimd.memset(ones_k[:, :], 1.0)
            gpsimd.drain()
            gpsimd.affine_select(
                idn[:, :],
                ones_k[:, :],
                [[1, K]],
                mybir.AluOpType.is_equal,
                0.0,
                base=0,
                channel_multiplier=-1,
            ).then_inc(s_idn, 1)
            gpsimd.wait_ge(s_dve, 2)
            for cc in range(6, NCORES):
                gather(gpsimd, cc, s_g2)

        @block.tensor
        def _(tensor):
            tensor.wait_ge(s_idn, 1)
            tensor.wait_ge(s_dve, 1)
            tensor.transpose(tp_ps[:, :], t1[:, :], idn[:, :]).then_inc(s_pe, 1)

        @block.vector
        def _(vector):
            vector.memset(tpad[:, :], float(SENT))
            vector.wait_ge(s_dw, 16)
            vector.wait_ge(s_dx, 32)
            # stage 1: tmp1[p, r, j] = xs[p, r + j] + w[j]
            vector.tensor_tensor(tmp1_w, xs_win, ws_b1, add)
            vector.drain()
            vector.tensor_reduce(
                t1[:, :], tmp1_w, axis=mybir.AxisListType.X, op=amax
            ).then_inc(s_dve, 1)
            vector.wait_ge(s_pe, 1)
            vector.drain()
            # tpad[r, 50 + p] = t1[p, r]
            vector.tensor_copy(tpad[0:S, PAD : PAD + K], tp_ps[:, :]).then_inc(
                s_dve, 1
            )
            vector.wait_ge(s_g, 96)
            vector.wait_ge(s_g2, 32)
            # stage 2: tmp2[P, c, i] = X[P, c + i] + w[i]
            vector.tensor_tensor(tmp2_w, X_win, ws_b2, add)
            vector.drain()
            vector.tensor_reduce(
                osb[:, :], tmp2_w, axis=mybir.AxisListType.X, op=amax
            ).then_inc(s_dve, 1)

    # restore normal barrier behavior for any framework-emitted code that
    # runs after the block (the skipped barriers are the init and block-exit
    # ones; the BSP postamble still drains all queues before NEFF end)
    nc._in_init = False
    return nc


def _prep_in_maps(input, scale):
    inp = np.asarray(input, dtype=np.float32)
    s = np.float32(np.asarray(scale).reshape(()))

    z = (np.arange(K, dtype=np.float32) - np.float32(PAD)).astype(np.float32)
    zsq = (z * z).astype(np.float32)
    wvec = (-zsq / (np.float32(4.0) * s)).astype(np.float32)
    w_rep = np.ascontiguousarray(np.tile(wvec[None, :], (NCORES * S, 1)))

    rowpad = np.full((K, XCOLS), SENT, dtype=np.float32)
    rowpad[:, PAD : PAD + K] = inp

    in_maps = []
    for k in range(NCORES):
        in_maps.append(
            {
                "x": np.ascontiguousarray(rowpad[:, S * k : S * k + WT]),
                "w": w_rep,
            }
        )
    return in_maps


def _unshard(results):
    out_full = np.empty((K, K), dtype=np.float32)
    for k, res in enumerate(results):
        o = np.asarray(res["out"]).reshape(NCORES, S, S)  # [cc, r_loc, c_in]
        block = o.transpose(1, 0, 2).reshape(S, NCORES * S)  # [r_loc, c]
        r0 = S * k
        nrows = min(S, K - r0)
        if nrows <= 0:
            continue
        out_full[r0 : r0 + nrows, :] = block[:nrows, :K]
    return out_full


def kernel(input, scale):
    from concourse.bass_utils import run_bass_kernel_spmd

    if "nc" not in _CACHE:
        _CACHE["nc"] = _build_nc()
    nc = _CACHE["nc"]

    in_maps = _prep_in_maps(input, scale)
    res = run_bass_kernel_spmd(nc, in_maps, core_ids=list(range(NCORES)))
    return _unshard(res.results)
